# revision 1
# baseline (speedup 1.0000x reference)
import numpy as np
import jax
import jax.numpy as jnp
from jax import lax
from functools import partial

HEADS = 4
NEG_SLOPE = 0.2
B, N, T, H, E, ED, KW = 1, 1000, 32, 128, 16000, 16, 3
D = H // HEADS
NCORES = 8
TL = T // NCORES  # 4 timesteps per core


def _layernorm(x, g, b, eps=1e-5):
    m = x.mean(-1, keepdims=True)
    v = ((x - m) ** 2).mean(-1, keepdims=True)
    return (x - m) * lax.rsqrt(v + eps) * g + b


def _prep_edges(edge_index, edge_attr):
    """Host-side static-topology preprocessing: self-loop attrs + padded
    per-node incoming-edge tables (turns segment ops into dense gathers)."""
    ei = np.asarray(edge_index).astype(np.int64)
    ea = np.asarray(edge_attr, np.float32)
    src0, dst0 = ei[0], ei[1]
    cnt = np.zeros(N, np.float32)
    np.add.at(cnt, dst0, 1.0)
    ssum = np.zeros((N, ED), np.float32)
    np.add.at(ssum, dst0, ea)
    loop_attr = ssum / np.maximum(cnt, 1.0)[:, None]
    eaF = np.concatenate([ea, loop_attr], 0)  # [E+N, ED]
    src = np.concatenate([src0, np.arange(N)])
    dst = np.concatenate([dst0, np.arange(N)])
    EP = E + N
    deg = np.zeros(N, np.int64)
    np.add.at(deg, dst, 1)
    Dmax = int(deg.max())
    order = np.argsort(dst, kind="stable")
    sdst = dst[order]
    starts = np.concatenate([[0], np.cumsum(deg)])
    pos = np.arange(EP) - starts[sdst]
    inc = np.zeros((N, Dmax), np.int64)
    mask = np.zeros((N, Dmax), np.float32)
    inc[sdst, pos] = order
    mask[sdst, pos] = 1.0
    src_inc = src[inc]  # [N, Dmax]
    return (eaF, src.astype(np.int32), dst.astype(np.int32),
            inc.astype(np.int32), mask, src_inc.astype(np.int32))


@partial(jax.pmap, in_axes=(0,) + (None,) * 16)
def _shard_fn(xh, eaF, src, dst, inc, mask, src_inc,
              conv_w, conv_b, ln1_g, ln1_b, Wl, Wr, We, att, gat_b,
              ln2gb):
    ln2_g, ln2_b = ln2gb[0], ln2gb[1]
    P = lax.Precision.HIGHEST
    # temporal conv over the haloed window (VALID on TL+2 -> TL outputs)
    xt = xh.transpose(0, 2, 1)  # [N, H, TL+2]
    y = lax.conv_general_dilated(xt, conv_w, (1,), 'VALID',
                                 dimension_numbers=('NCH', 'OIH', 'NCH'))
    y = y + conv_b[None, :, None]
    y = y.transpose(0, 2, 1)  # [N, TL, H]
    x1 = _layernorm(xh[:, 1:TL + 1, :] + y, ln1_g, ln1_b)  # [N, TL, H]

    xs = x1.transpose(1, 0, 2)  # [TL, N, H]
    ee = (eaF @ We).reshape(-1, HEADS, D)  # [EP, K, D]

    def graph(xg):
        gl = jnp.matmul(xg, Wl, precision=P).reshape(N, HEADS, D)
        gr = jnp.matmul(xg, Wr, precision=P).reshape(N, HEADS, D)
        s = jax.nn.leaky_relu(gl[src] + gr[dst] + ee, NEG_SLOPE)
        logits = jnp.einsum('ekd,kd->ek', s, att, precision=P)  # [EP, K]
        L = logits[inc]  # [N, Dmax, K]
        L = jnp.where(mask[..., None] > 0, L, -1e30)
        mx = L.max(1, keepdims=True)
        ex = jnp.exp(L - mx) * mask[..., None]
        den = ex.sum(1)  # [N, K]
        vals = gl[src_inc]  # [N, Dmax, K, D]
        aggr = (ex[..., None] * vals).sum(1) / den[..., None]
        return aggr.reshape(N, H) + gat_b

    outg = jax.vmap(graph)(xs)  # [TL, N, H]
    return _layernorm(x1 + outg.transpose(1, 0, 2), ln2_g, ln2_b)


_CACHE = {}


def _edge_state(edge_index, edge_attr):
    import hashlib
    k = hashlib.md5(np.ascontiguousarray(edge_index).tobytes()
                    + np.ascontiguousarray(edge_attr).tobytes()).hexdigest()
    if k not in _CACHE:
        eaF, src, dst, inc, mask, src_inc = _prep_edges(edge_index, edge_attr)
        _CACHE.clear()
        _CACHE[k] = tuple(jnp.asarray(a) for a in
                          (eaF, src, dst, inc, mask, src_inc))
    return _CACHE[k]


def kernel(**inputs):
    x = np.asarray(inputs['x'], np.float32)
    eaF, src, dst, inc, mask, src_inc = _edge_state(
        inputs['edge_index'], inputs['edge_attr'])

    xp = np.pad(x[0], ((0, 0), (1, 1), (0, 0)))  # [N, T+2, H]
    shards = np.stack([xp[:, s * TL:s * TL + TL + 2, :]
                       for s in range(NCORES)], 0)  # [8, N, TL+2, H]

    out = _shard_fn(
        jnp.asarray(shards), eaF, src, dst, inc, mask, src_inc,
        jnp.asarray(np.asarray(inputs['conv_w'], np.float32)),
        jnp.asarray(np.asarray(inputs['conv_b'], np.float32)),
        jnp.asarray(np.asarray(inputs['ln1_g'], np.float32)),
        jnp.asarray(np.asarray(inputs['ln1_b'], np.float32)),
        jnp.asarray(np.asarray(inputs['Wl'], np.float32)),
        jnp.asarray(np.asarray(inputs['Wr'], np.float32)),
        jnp.asarray(np.asarray(inputs['We'], np.float32)),
        jnp.asarray(np.asarray(inputs['att'], np.float32)),
        jnp.asarray(np.asarray(inputs['gat_b'], np.float32)),
        jnp.stack([np.asarray(inputs['ln2_g'], np.float32),
                   np.asarray(inputs['ln2_b'], np.float32)]),
    )  # [8, N, TL, H]
    out = np.asarray(out)  # [8, N, TL, H]
    full = out.transpose(1, 0, 2, 3).reshape(N, T, H)[None]
    return full.astype(np.float32)

